# revision 1
# baseline (speedup 1.0000x reference)
"""Trainium2 Bass kernel for DynamicEdgeConstruction (top-k masked softmax
attention matrix).

Computes, for x [B=4, N=4096, C=256], W_q/W_k [256, 64]:
    Q = x @ W_q; K = x @ W_k
    S = Q K^T / sqrt(64)           [B, N, N]
    A = softmax over the top-k entries of each row of S, zeros elsewhere.

Sharding: 8 NeuronCores, 2 per batch element, each handling 2048 query rows
(row-wise sequence parallel; K replicated per batch).

Device algorithm per core (dense formulation):
  - S row-tile [128, 4096] via fp32 PE matmuls into PSUM (softmax scale folded
    into W_q host-side; 1/8 is a power of two so this is bit-exact).
  - ACT copies S to SBUF; DVE max8 gives the exact top-8 values per row.
  - Tiny softmax over the top-k values yields r = 1/Z per row.
  - maskr = (S >= t_k) * r  (one DVE tensor_scalar pass, per-row scalars).
  - E = exp(S - m)           (one ACT pass, per-row bias).
  - A = E * maskr            (tensor_tensor; split DVE/GpSimd for balance).
  - Plain DMA of the dense tile to the output.
"""

import numpy as np

B, N, C, DK = 4, 4096, 256, 64
NCORES = 8
RPC = N // 2          # rows per core (2048)
P = 128               # partitions
NT = RPC // P         # row tiles per core (16)
CHUNK = 512           # matmul free-dim chunk (one PSUM bank fp32)
HALF = 2048           # S half-tile free size (4 PSUM banks)

# which row-tiles run the final multiply on DVE (rest go to GpSimd)
DVE_MULT_TILES = frozenset()

# use float32r (fast fp32 mode) for PE matmuls
F32R = False

_cache = {}


def _build(k: int, f32r: bool = False, bench_reps: int | None = None,
           probe: str | None = None):
    probes = set((probe or "").split("+")) - {""}
    """Build + compile the SPMD Bass program for top-k = k (1..8)."""
    import concourse.bass as bass
    import concourse.bacc as bacc
    import concourse.tile as tile
    import concourse.mybir as mybir
    from contextlib import ExitStack

    f32 = mybir.dt.float32
    mmdt = (lambda ap: ap.bitcast(mybir.dt.float32r)) if f32r else (lambda ap: ap)

    nc = bacc.Bacc("TRN2", target_bir_lowering=False, debug=False,
                   num_devices=NCORES)

    xkT_d = nc.dram_tensor("xkT", [C, N], f32, kind="ExternalInput").ap()
    xqT_d = nc.dram_tensor("xqT", [C, RPC], f32, kind="ExternalInput").ap()
    wq_d = nc.dram_tensor("wq", [C, DK], f32, kind="ExternalInput").ap()
    wk_d = nc.dram_tensor("wk", [C, DK], f32, kind="ExternalInput").ap()
    out_d = nc.dram_tensor("out", [RPC, N], f32, kind="ExternalOutput").ap()

    with tile.TileContext(nc) as tc:
        with ExitStack() as ctx:
            const = ctx.enter_context(tc.tile_pool(name="const", bufs=1))

            xkT = [const.tile([P, N], f32, tag=f"xkT{i}", name=f"xkT{i}")
                   for i in range(2)]
            xqT = [const.tile([P, RPC], f32, tag=f"xqT{i}", name=f"xqT{i}")
                   for i in range(2)]
            wq = [const.tile([P, DK], f32, tag=f"wq{i}", name=f"wq{i}")
                  for i in range(2)]
            wk = [const.tile([P, DK], f32, tag=f"wk{i}", name=f"wk{i}")
                  for i in range(2)]
            KT = const.tile([DK, N], f32, tag="KT")
            QT = const.tile([DK, RPC], f32, tag="QT")

            # spread input loads over independent DMA queues; small weight
            # tiles first so they never queue behind the 2 MiB x loads
            nc.sync.dma_start(wk[0][:], wk_d[0:P, :])
            nc.scalar.dma_start(wk[1][:], wk_d[P:2 * P, :])
            nc.gpsimd.dma_start(wq[0][:], wq_d[0:P, :])
            nc.gpsimd.dma_start(wq[1][:], wq_d[P:2 * P, :])
            nc.sync.dma_start(xkT[0][:], xkT_d[0:P, :])
            nc.scalar.dma_start(xkT[1][:], xkT_d[P:2 * P, :])
            nc.gpsimd.dma_start(xqT[0][:], xqT_d[0:P, :])
            nc.sync.dma_start(xqT[1][:], xqT_d[P:2 * P, :])

            # Projections: KT = wk^T @ xkT, QT = wq^T @ xqT (contraction over
            # C = 256 in two accumulating halves). Only KT + the first QT
            # chunk happen up front; later QT chunks are interleaved into the
            # main loop (borrowing an S-PSUM slot) so the pipeline starts
            # ~15 us sooner.
            def proj_into(pool, tag, dst, w, src, sl):
                pt = pool.tile([DK, CHUNK], f32, tag=tag, name="pt")
                nc.tensor.matmul(out=pt[:], lhsT=mmdt(w[0][:]),
                                 rhs=mmdt(src[0][:, sl]),
                                 start=True, stop=False)
                nc.tensor.matmul(out=pt[:], lhsT=mmdt(w[1][:]),
                                 rhs=mmdt(src[1][:, sl]),
                                 start=False, stop=True)
                nc.scalar.copy(dst[:, sl], pt[:])

            with tc.tile_pool(name="proj_ps", bufs=2, space="PSUM") as proj_ps:
                for ch in range(N // CHUNK):
                    proj_into(proj_ps, "proj", KT, wk, xkT,
                              slice(ch * CHUNK, (ch + 1) * CHUNK))
                proj_into(proj_ps, "proj", QT, wq, xqT, slice(0, CHUNK))

            spool = ctx.enter_context(tc.tile_pool(name="ssb", bufs=3))
            mpool = ctx.enter_context(tc.tile_pool(name="mp", bufs=3))
            small = ctx.enter_context(tc.tile_pool(name="small", bufs=5))
            # One PSUM pool; per tile, slot a holds h0 (freed quickly by the
            # copy), slot b holds h1 (freed by the exp that reads it).
            sps = ctx.enter_context(tc.tile_pool(name="sps", bufs=2, space="PSUM"))
            MCUT = 3584   # gpsimd multiplies [0:MCUT), DVE takes the rest

            # Two-stage emission so each engine's in-order stream never
            # interleaves this tile's late ops before next tile's early ops:
            #   stage A(t):  matmuls, copy h0, max8 (h0 sbuf + h1 psum), negm
            #   stage B(t-1): tiny softmax, exp h0/h1, maskr-on-E, mult, DMA
            state = {}

            QCHUNK_TILES = CHUNK // P   # S-tiles covered per QT chunk (4)

            def stage_a(t):
                if t >= QCHUNK_TILES - 1 and (t + 1) % QCHUNK_TILES == 0:
                    nq = (t + 1) // QCHUNK_TILES   # QT chunk for tiles t+1..t+4
                    if nq < RPC // CHUNK:
                        proj_into(sps, "sps", QT, wq, xqT,
                                  slice(nq * CHUNK, (nq + 1) * CHUNK))
                s_sb = spool.tile([P, N], f32, tag="s_sb", name="s_sb")
                lhsT = QT[:, t * P:(t + 1) * P]
                pa = sps.tile([P, HALF], f32, tag="sps", name="pa")
                pb = sps.tile([P, HALF], f32, tag="sps", name="pb")
                nch = (HALF // CHUNK) // (2 if "halfpe" in probes else 1)
                for h, ps in ((0, pa), (1, pb)):
                    for ch in range(nch):
                        psl = slice(ch * CHUNK, (ch + 1) * CHUNK)
                        ksl = slice(h * HALF + ch * CHUNK,
                                    h * HALF + (ch + 1) * CHUNK)
                        nc.tensor.matmul(out=ps[:, psl], lhsT=mmdt(lhsT),
                                         rhs=mmdt(KT[:, ksl]),
                                         start=True, stop=True)
                nc.scalar.copy(s_sb[:, 0:HALF], pa[:])
                if "fullcopy" in probes:
                    nc.scalar.copy(s_sb[:, HALF:N], pb[:])

                V2 = small.tile([P, 16], f32, tag="V2", name="V2")
                V = small.tile([P, 8], f32, tag="V", name="V")
                if "nomax" in probes:
                    nc.vector.memset(V[:], 1.0)
                elif "fullcopy" in probes:
                    nc.vector.max(V[:], s_sb[:])
                else:
                    nc.vector.max(V2[:, 0:8], s_sb[:, 0:HALF])
                    nc.vector.max(V2[:, 8:16], pb[:])
                    nc.vector.max(V[:], V2[:])
                negm = small.tile([P, 1], f32, tag="negm", name="negm")
                nc.vector.tensor_scalar_mul(negm[:], V[:, 0:1], -1.0)
                if k < 8:
                    nc.vector.memset(V[:, k:8], -1e30)
                state[t] = (s_sb, pb, V, negm)

            def stage_b(t):
                s_sb, pb, V, negm = state.pop(t)
                E8 = small.tile([P, 8], f32, tag="E8", name="E8")
                Z = small.tile([P, 1], f32, tag="Z", name="Z")
                nc.scalar.activation(E8[:], V[:],
                                     mybir.ActivationFunctionType.Exp,
                                     bias=negm[:, 0:1], scale=1.0,
                                     accum_out=Z[:])
                r = small.tile([P, 1], f32, tag="r", name="r")
                nc.vector.reciprocal(r[:], Z[:])

                # E = exp(S - m): h0 in place in SBUF, h1 straight from PSUM
                nc.scalar.activation(s_sb[:, 0:HALF], s_sb[:, 0:HALF],
                                     mybir.ActivationFunctionType.Exp,
                                     bias=negm[:, 0:1], scale=1.0)
                h1_src = s_sb[:, HALF:N] if "fullcopy" in probes else pb[:]
                nc.scalar.activation(s_sb[:, HALF:N], h1_src,
                                     mybir.ActivationFunctionType.Exp,
                                     bias=negm[:, 0:1], scale=1.0)

                # maskr = (E >= e_k) * r  — exact same exp images on both
                # sides of the compare, so selection stays consistent. Halved
                # so the multiply can start on h0 while h1's compare runs.
                maskr = mpool.tile([P, N], f32, tag="maskr", name="maskr")
                for sl in (slice(0, HALF), slice(HALF, N)):
                    nc.vector.tensor_scalar(maskr[:, sl], s_sb[:, sl],
                                            E8[:, k - 1:k], r[:, 0:1],
                                            op0=mybir.AluOpType.is_ge,
                                            op1=mybir.AluOpType.mult)

                # A = E * maskr (in place over maskr): bulk on GpSimd in two
                # chunks, small slice on DVE at the end of its stream.
                if "nomult" not in probes:
                    nc.gpsimd.tensor_tensor(maskr[:, 0:HALF], s_sb[:, 0:HALF],
                                            maskr[:, 0:HALF],
                                            op=mybir.AluOpType.mult)
                    nc.gpsimd.tensor_tensor(maskr[:, HALF:MCUT],
                                            s_sb[:, HALF:MCUT],
                                            maskr[:, HALF:MCUT],
                                            op=mybir.AluOpType.mult)
                    nc.vector.tensor_tensor(maskr[:, MCUT:N], s_sb[:, MCUT:N],
                                            maskr[:, MCUT:N],
                                            op=mybir.AluOpType.mult)

                nc.sync.dma_start(out_d[t * P:(t + 1) * P, :], maskr[:])

            def main_loop():
                for t in range(NT + 1):
                    if t < NT:
                        stage_a(t)
                    if t >= 1:
                        stage_b(t - 1)

            if bench_reps is None:
                main_loop()
            else:
                # benchmark mode: repeat the whole compute on-device so real
                # HW time is measurable through the (transfer-dominated) wall
                nbody = 2 if "body2" in probes else 1
                with tc.For_i(0, bench_reps, 1):
                    for _ in range(nbody):
                        main_loop()

    nc.compile()
    return nc


def _get_program(k: int):
    if k not in _cache:
        _cache[k] = _build(k, f32r=F32R)
    return _cache[k]


def kernel(x, W_q, W_k, top_k):
    from concourse.bass_utils import run_bass_kernel_spmd

    x = np.asarray(x, dtype=np.float32)
    W_q = np.asarray(W_q, dtype=np.float32)
    W_k = np.asarray(W_k, dtype=np.float32)
    k = int(np.asarray(top_k))
    assert x.shape == (B, N, C) and W_q.shape == (C, DK) and W_k.shape == (C, DK)
    assert 1 <= k <= 8, f"top_k={k} unsupported"

    nc = _get_program(k)

    wq_scaled = np.ascontiguousarray(W_q * np.float32(DK) ** np.float32(-0.5),
                                     dtype=np.float32)
    wk_c = np.ascontiguousarray(W_k, dtype=np.float32)

    in_maps = []
    for c in range(NCORES):
        b, half = c // 2, c % 2
        xT = np.ascontiguousarray(x[b].T)                      # [C, N]
        xqT = np.ascontiguousarray(xT[:, half * RPC:(half + 1) * RPC])
        in_maps.append({"xkT": xT, "xqT": xqT, "wq": wq_scaled, "wk": wk_c})

    res = run_bass_kernel_spmd(nc, in_maps, list(range(NCORES)))

    A = np.empty((B, N, N), dtype=np.float32)
    for c in range(NCORES):
        b, half = c // 2, c % 2
        A[b, half * RPC:(half + 1) * RPC, :] = res.results[c]["out"]
    return A



# revision 6
# speedup vs baseline: 1.3379x; 1.3379x over previous
"""Trainium2 Bass kernel v3 for DynamicEdgeConstruction (top-k masked softmax).

Computes, for x [B=4, N=4096, C=256], W_q/W_k [256, 64]:
    Q = x @ W_q; K = x @ W_k
    S = Q K^T / sqrt(64)           [B, N, N]
    A = softmax over the top-k entries of each row of S, zeros elsewhere.

Sharding: 8 NeuronCores, 2 per batch element, each handling 2048 query rows.
Each core receives the full xT [C, N] for its batch element, with columns
rotated so that its own query block is always columns [0:2048] (the host
un-rotates the output columns when reassembling).

Device pipeline, per 128-row tile, processed as two 2048-column half-tiles:
  - each half's S lands in a 4-bank PSUM tile (the two halves double-buffer
    the 8 PSUM banks, so tile t+1's matmuls overlap tile t's post-processing)
  - the only PSUM readers are the PSUM->SBUF copies (split ACT/Pool), so the
    banks recycle after ~mm+copy instead of holding through the whole
    max8 -> logZ -> exp chain
  - DVE max8 per half from SBUF, merged top-8 V; ACT E8=exp(V) accum -> Z;
    DVE reciprocal; ACT Ln(1/Z) = -logZ (exp+ln+copy share one pinned
    activation table)
  - ACT exp(S - logZ) SBUF->SBUF into the output tile
  - fused scalar_tensor_tensor computes A = (S >= t_k) * E per slice
    (split DVE/Pool); t_k = V[k-1] compares in the exact S domain
  - dense 2 MiB DMA per tile to the output
"""

import numpy as np

B, N, C, DK = 4, 4096, 256, 64
NCORES = 8
RPC = N // 2          # rows per core (2048)
P = 128               # partitions
NT = RPC // P         # row tiles per core (16)
CHUNK = 512           # matmul free-dim chunk (one PSUM bank fp32)
HALF = 2048           # half-tile columns (4 PSUM banks)

# elementwise split points (tunable): per half of 2048, columns [0:ACT_COPY)
# copied by ACT, [ACT_COPY:ACT_COPY+DVE_COPY) by DVE, rest by DMA (GpSimd
# cannot touch PSUM on real hardware). stt columns [0:STT_DVE) on DVE, rest
# on Pool.
ACT_COPY = 1776      # per half (of 2048)
STT_DVE = 1984       # of 4096: fused stt on DVE; Pool does cmp+mult on rest

F32R = True

_cache = {}


def _build(k: int, f32r: bool = F32R):
    """Build + compile the SPMD Bass program for top-k = k (1..8)."""
    import concourse.bass as bass
    import concourse.bacc as bacc
    import concourse.tile as tile
    import concourse.mybir as mybir
    from contextlib import ExitStack

    f32 = mybir.dt.float32
    kqdt = mybir.dt.float32r if f32r else f32

    nc = bacc.Bacc("TRN2", target_bir_lowering=False, debug=False,
                   num_devices=NCORES)

    xT_d = nc.dram_tensor("xT", [C, N], f32, kind="ExternalInput").ap()
    wq_d = nc.dram_tensor("wq", [C, DK], f32, kind="ExternalInput").ap()
    wk_d = nc.dram_tensor("wk", [C, DK], f32, kind="ExternalInput").ap()
    out_d = nc.dram_tensor("out", [RPC, N], f32, kind="ExternalOutput").ap()

    Exp = mybir.ActivationFunctionType.Exp
    Ln = mybir.ActivationFunctionType.Ln

    with tile.TileContext(nc) as tc:
        with ExitStack() as ctx:
            # Pin the activation table that holds exp+ln+copy so Exp/Ln
            # alternation never reloads tables.
            from concourse.hw_specs import get_activation_tables
            tables = list(get_activation_tables(nc.m.arch).items())
            tid = None
            for i, (nm, funcs) in enumerate(tables):
                names = {f.name for f in funcs}
                if "Exp" in names and "Ln" in names:
                    tid = i
                    break
            assert tid is not None, "no act table with Exp+Ln"
            nc.scalar.add_instruction(mybir.InstLoadActFuncSet(
                name=nc.get_next_instruction_name(), ins=[], outs=[],
                act_func_set_id=tid))

            const = ctx.enter_context(tc.tile_pool(name="const", bufs=1))

            xT = [const.tile([P, N], f32, tag=f"xT{i}", name=f"xT{i}")
                  for i in range(2)]
            wq = [const.tile([P, DK], f32, tag=f"wq{i}", name=f"wq{i}")
                  for i in range(2)]
            wk = [const.tile([P, DK], f32, tag=f"wk{i}", name=f"wk{i}")
                  for i in range(2)]
            KT = const.tile([DK, N], kqdt, tag="KT")
            QT = const.tile([DK, RPC], kqdt, tag="QT")

            # weights first (tiny), then x in column blocks (query/key block
            # [0:2048] of both partition halves first)
            nc.sync.dma_start(wk[0][:], wk_d[0:P, :])
            nc.scalar.dma_start(wk[1][:], wk_d[P:2 * P, :])
            nc.gpsimd.dma_start(wq[0][:], wq_d[0:P, :])
            nc.gpsimd.dma_start(wq[1][:], wq_d[P:2 * P, :])
            nc.sync.dma_start(xT[0][:, 0:RPC], xT_d[0:P, 0:RPC])
            nc.scalar.dma_start(xT[1][:, 0:RPC], xT_d[P:2 * P, 0:RPC])
            nc.sync.dma_start(xT[0][:, RPC:N], xT_d[0:P, RPC:N])
            nc.scalar.dma_start(xT[1][:, RPC:N], xT_d[P:2 * P, RPC:N])

            # Projections: KT = wk^T @ xT (all N cols), QT = wq^T @ xT[:,0:RPC]
            # (contraction over C=256 in two accumulating halves, fp32).
            # The PSUM->SBUF copy writes float32r, which both rounds the
            # operands for the fp32r S-matmuls and satisfies birverifier.
            def proj_into(pool, dst, w, sl, tag="proj"):
                pt = pool.tile([DK, CHUNK], f32, tag=tag, name="pt")
                nc.tensor.matmul(out=pt[:], lhsT=w[0][:], rhs=xT[0][:, sl],
                                 start=True, stop=False)
                nc.tensor.matmul(out=pt[:], lhsT=w[1][:], rhs=xT[1][:, sl],
                                 start=False, stop=True)
                nc.scalar.copy(dst[:, sl], pt[:])

            spool = ctx.enter_context(tc.tile_pool(name="ssb", bufs=4))
            mpool = ctx.enter_context(tc.tile_pool(name="mp", bufs=4))
            mskpool = ctx.enter_context(tc.tile_pool(name="msk", bufs=2))
            small = ctx.enter_context(tc.tile_pool(name="small", bufs=8))

            state = {}

            def stage_a(t, h, pool=None, tag="ps"):
                """matmul + copies + max8 for half h of tile t."""
                lhsT = QT[:, t * P:(t + 1) * P]
                ps = (pool or sps).tile([P, HALF], f32, tag=tag, name="ps",
                                        bufs=(1 if pool is not None else None))
                base = h * HALF
                for ch in range(HALF // CHUNK):
                    psl = slice(ch * CHUNK, (ch + 1) * CHUNK)
                    ksl = slice(base + ch * CHUNK, base + (ch + 1) * CHUNK)
                    nc.tensor.matmul(out=ps[:, psl], lhsT=lhsT,
                                     rhs=KT[:, ksl], start=True, stop=True)

                if h == 0:
                    s_sb = spool.tile([P, N], f32, tag="s_sb", name="s_sb")
                    V2 = small.tile([P, 16], f32, tag="V2", name="V2")
                    state[t] = [s_sb, V2, None]
                else:
                    s_sb, V2, _ = state[t]
                # PSUM -> SBUF copies are the only PSUM readers
                nc.scalar.copy(s_sb[:, base:base + ACT_COPY],
                               ps[:, 0:ACT_COPY])
                nc.vector.tensor_scalar(
                    s_sb[:, base + ACT_COPY:base + HALF],
                    ps[:, ACT_COPY:HALF], 1.0, None,
                    op0=mybir.AluOpType.mult)
                nc.vector.max(V2[:, 8 * h:8 * h + 8],
                              s_sb[:, base:base + HALF])

            def stage_b1(t):
                s_sb, V2, _ = state[t]
                V = small.tile([P, 8], f32, tag="V", name="V")
                nc.vector.max(V[:], V2[:])
                if k < 8:
                    nc.vector.memset(V[:, k:8], -1e30)
                E8 = small.tile([P, 8], f32, tag="E8", name="E8")
                Z = small.tile([P, 1], f32, tag="Z", name="Z")
                r = small.tile([P, 1], f32, tag="r", name="r")
                negc = small.tile([P, 1], f32, tag="negc", name="negc")
                nc.scalar.activation(E8[:], V[:], Exp, accum_out=Z[:])
                nc.vector.reciprocal(r[:], Z[:])
                nc.scalar.activation(negc[:], r[:], Ln)
                state[t] = [s_sb, V, negc]

            def stage_b2_exp(t, h):
                s_sb, V, negc, m_sb = state2[t]
                base = h * HALF
                nc.scalar.activation(m_sb[:, base:base + HALF],
                                     s_sb[:, base:base + HALF], Exp,
                                     bias=negc[:, 0:1], scale=1.0)

            def stage_b2_fin(t):
                s_sb, V, negc, m_sb = state2.pop(t)
                # A = (S >= t_k) * E fused per slice; t_k = V[k-1] compares
                # in the exact S domain
                tk = V[:, k - 1:k]
                nc.vector.scalar_tensor_tensor(
                    m_sb[:, 0:STT_DVE], s_sb[:, 0:STT_DVE], tk,
                    m_sb[:, 0:STT_DVE],
                    op0=mybir.AluOpType.is_ge, op1=mybir.AluOpType.mult)
                # Pool cannot run scalar_tensor_tensor: two-pass select
                # (mask via tensor_scalar, then in-place tensor_tensor mult)
                msk = mskpool.tile([P, N - STT_DVE], f32, tag="msk",
                                   name="msk")
                nc.gpsimd.tensor_scalar(msk[:], s_sb[:, STT_DVE:N],
                                        tk, None,
                                        op0=mybir.AluOpType.is_ge)
                nc.gpsimd.tensor_tensor(m_sb[:, STT_DVE:N],
                                        m_sb[:, STT_DVE:N], msk[:],
                                        op=mybir.AluOpType.mult)

                nc.sync.dma_start(out_d[t * P:(t + 1) * P, :], m_sb[:])

            state2 = {}

            # fill: project QT chunk 0 and the KT half needed by tile-0
            # h0, run tile-0 h0 immediately (PSUM from the proj scope), then
            # the rest of KT and tile-0 h1.
            with tc.tile_pool(name="proj_ps", bufs=2, space="PSUM") as pps:
                proj_into(pps, QT, wq, slice(0, CHUNK))
                for ch in range(RPC // CHUNK):
                    proj_into(pps, KT, wk, slice(ch * CHUNK, (ch + 1) * CHUNK))
                stage_a(0, 0, pool=pps, tag="big")
                for ch in range(RPC // CHUNK, N // CHUNK):
                    proj_into(pps, KT, wk, slice(ch * CHUNK, (ch + 1) * CHUNK))
                stage_a(0, 1, pool=pps, tag="big")

            # one PSUM tag, 2 bufs of 4 banks: half-tiles rotate through them
            sps = ctx.enter_context(tc.tile_pool(name="sps", bufs=2,
                                                 space="PSUM"))

            # emission order per iteration fixes each engine's instruction
            # order: DVE [merge(t-1), recip(t-1), max8-h0(t), max8-h1(t),
            # stt(t-2)]; ACT [E8(t-1), Ln(t-1), copyA-h0(t), exp0(t-2),
            # copyA-h1(t), exp1(t-2)]; Pool [copyP-h0(t), copyP-h1(t),
            # stt(t-2)]
            for t in range(1, NT + 2):
                if 1 <= t <= NT:
                    stage_b1(t - 1)
                    s_sb, V, negc = state.pop(t - 1)
                    m_sb = mpool.tile([P, N], f32, tag="m_sb", name="m_sb")
                    state2[t - 1] = [s_sb, V, negc, m_sb]
                if 1 <= t <= 3:
                    # late QT chunks, borrowing an sps slot for proj PSUM
                    proj_into(sps, QT, wq, slice(t * CHUNK, (t + 1) * CHUNK),
                              tag="ps")
                if t < NT:
                    stage_a(t, 0)
                if t >= 2:
                    stage_b2_exp(t - 2, 0)
                if t < NT:
                    stage_a(t, 1)
                if t >= 2:
                    stage_b2_exp(t - 2, 1)
                    stage_b2_fin(t - 2)

    nc.compile()
    return nc


def _get_program(k: int):
    if k not in _cache:
        _cache[k] = _build(k)
    return _cache[k]


def kernel(x, W_q, W_k, top_k):
    from concourse.bass_utils import run_bass_kernel_spmd

    x = np.asarray(x, dtype=np.float32)
    W_q = np.asarray(W_q, dtype=np.float32)
    W_k = np.asarray(W_k, dtype=np.float32)
    k = int(np.asarray(top_k))
    assert x.shape == (B, N, C) and W_q.shape == (C, DK) and W_k.shape == (C, DK)
    assert 1 <= k <= 8, f"top_k={k} unsupported"

    nc = _get_program(k)

    wq_scaled = np.ascontiguousarray(W_q * np.float32(DK) ** np.float32(-0.5),
                                     dtype=np.float32)
    wk_c = np.ascontiguousarray(W_k, dtype=np.float32)

    in_maps = []
    for c in range(NCORES):
        b, half = c // 2, c % 2
        xT = np.ascontiguousarray(x[b].T)                      # [C, N]
        if half == 1:
            # rotate so this core's query block sits at columns [0:RPC]
            xT = np.ascontiguousarray(
                np.concatenate([xT[:, RPC:], xT[:, :RPC]], axis=1))
        in_maps.append({"xT": xT, "wq": wq_scaled, "wk": wk_c})

    res = run_bass_kernel_spmd(nc, in_maps, list(range(NCORES)))

    A = np.empty((B, N, N), dtype=np.float32)
    for c in range(NCORES):
        b, half = c // 2, c % 2
        r = res.results[c]["out"]
        rows = slice(half * RPC, (half + 1) * RPC)
        if half == 0:
            A[b, rows, :] = r
        else:
            # un-rotate the key columns
            A[b, rows, RPC:N] = r[:, 0:RPC]
            A[b, rows, 0:RPC] = r[:, RPC:N]
    return A
